# revision 16
# baseline (speedup 1.0000x reference)
"""Trainium2 Bass kernel for AttributeGNN message passing.

Math (reference):
    cat  = [img broadcast, edge]                      # [B, A, 2D]
    agg  = cat @ W_agg.T + b_agg                      # [B, A, D]
    attr = einsum('bao,aoj->baj', agg, P_fwd)         # output 1
    ind  = sum_a relu(attr @ P_bwd[a]) * w_a          # output 2  [B, D]

Kernel restructuring:
    agg[b,a,:]  = img_lin[b,:] + edge[b,a,:] @ W2.T   (W1 = W_agg[:,:D], W2 = W_agg[:,D:])
    img_lin     = img @ W1.T + b_agg                  (computed once per batch tile)
    attr        = agg @ P_fwd[a]
    z           = attr @ P_bwd[a] = agg @ (P_fwd[a] @ P_bwd[a])   <- fused on host (fp64)
    attr and z come out of ONE matmul with rhs = [P_fwd[a] | P_comb[a]]  (N=512)

Layout: all activations enter the PE feature-major. The host pre-transposes
edge/img (batch-sharded across 8 cores), so the device does zero transposes.
agg is produced feature-major [o, b] in PSUM, moved to SBUF (fused with the
img_lin add on DVE), then used as the *stationary* operand so attr/z come out
batch-major [b, j] and can be DMA'd straight to the batch-major outputs.
"""

import contextlib

import numpy as np

import concourse.bass as bass
import concourse.mybir as mybir
import concourse.tile as tile
from concourse import bacc
from concourse.alu_op_type import AluOpType
from concourse.bass_utils import run_bass_kernel_spmd

B, A, D = 16384, 16, 256
N_CORES = 8
BL = B // N_CORES          # 2048 batch rows per core
BT = 512                   # batch tile (matmul moving free dim)
NBT = BL // BT             # 4 batch tiles per core
NG = BT // 128             # 4 partition groups per batch tile
OC = D // 128              # 2 chunks of 128 over the feature dim
F32 = mybir.dt.float32
F32R = mybir.dt.float32r

_prog_cache = {}


def _build_program(wmode: str, repeat=None):
    """Build + compile the per-core SPMD program (identical on all cores).

    repeat: if set, wrap the whole body in a hardware For_i loop that
    recomputes the same outputs `repeat` times (benchmarking only — lets
    wall-clock timing resolve HW time above the ~80ms dispatch overhead).
    """
    nc = bacc.Bacc("TRN2", target_bir_lowering=False, debug=False,
                   num_devices=N_CORES)

    edge_T = nc.dram_tensor("edge_T", [A, D, BL], F32R, kind="ExternalInput").ap()
    img_T = nc.dram_tensor("img_T", [D, BL], F32R, kind="ExternalInput").ap()
    w1t = nc.dram_tensor("w1t", [D, D], F32R, kind="ExternalInput").ap()
    w2t = nc.dram_tensor("w2t", [D, D], F32R, kind="ExternalInput").ap()
    rhs_cat = nc.dram_tensor("rhs_cat", [A, D, 2 * D], F32R, kind="ExternalInput").ap()
    bias2 = nc.dram_tensor("bias2", [128, OC], F32, kind="ExternalInput").ap()
    wvec = nc.dram_tensor("wvec", [128, A], F32, kind="ExternalInput").ap()
    attr_o = nc.dram_tensor("attr_out", [BL, A, D], F32, kind="ExternalOutput").ap()
    ind_o = nc.dram_tensor("ind_out", [BL, D], F32, kind="ExternalOutput").ap()

    # wmode: "ones" (w == 1: relu+accumulate fused into one DVE op),
    # "nonneg" (relu+scale on ACT for half the groups), "general".
    act_groups = (0, 1) if wmode == "nonneg" else ()

    # fp32 matmul is 4 cycles/col on the PE; float32r streams at 1 cycle/col
    # when the moving free dim is >= 256. Everything feeding the PE is
    # declared float32r (same 4-byte numpy representation host-side).

    with tile.TileContext(nc) as tc:
        with (
            tc.tile_pool(name="const", bufs=1) as const,
            tc.tile_pool(name="efm", bufs=2) as efm_pool,
            tc.tile_pool(name="imgfm", bufs=2) as imgfm_pool,
            tc.tile_pool(name="imglin", bufs=2) as imglin_pool,
            tc.tile_pool(name="agg", bufs=4) as agg_pool,
            tc.tile_pool(name="attr", bufs=2) as attr_pool,
            tc.tile_pool(name="tmp", bufs=2) as tmp_pool,
            tc.tile_pool(name="acc", bufs=2) as acc_pool,
            tc.tile_pool(name="ps", bufs=3, space="PSUM") as ps_pool,
            tc.tile_pool(name="outps", bufs=5, space="PSUM") as outps_pool,
        ):
            # ---- resident constants ----
            w1t_sb = const.tile([128, OC, D], F32R)
            nc.sync.dma_start(w1t_sb[:], w1t.rearrange("(kc p) o -> p kc o", p=128))
            w2t_sb = const.tile([128, OC, D], F32R)
            nc.sync.dma_start(w2t_sb[:], w2t.rearrange("(ic p) o -> p ic o", p=128))
            # per-quad weight chunks are DMA'd just-in-time inside the first
            # batch tile's a-loop (an upfront 8.4MB load would serialize
            # ~23us of DMA before any compute can start)
            rhs_sb = [const.tile([128, 4, OC, 2 * D], F32R, tag=f"rhs{q}",
                                 name=f"rhs_sb{q}")
                      for q in range(A // 4)]
            bias_sb = const.tile([128, OC], F32)
            nc.sync.dma_start(bias_sb[:], bias2[:])
            wvec_sb = const.tile([128, A], F32)
            nc.sync.dma_start(wvec_sb[:], wvec[:])

            if repeat is not None:
                # bench variant: hoist the per-quad weight loads so the
                # repeated body excludes them
                for q in range(A // 4):
                    nc.sync.dma_start(
                        rhs_sb[q][:],
                        rhs_cat[q * 4:q * 4 + 4].rearrange(
                            "a (oc p) n -> p a oc n", p=128),
                    )
                loop_cm = tc.For_i(0, repeat, 1)
            else:
                loop_cm = contextlib.nullcontext()
            with loop_cm:
              for bt in range(NBT):
                  b0 = bt * BT
                  # ---- img_lin for this batch tile (shared across all a) ----
                  img_fm = imgfm_pool.tile([128, OC, BT], F32R)
                  nc.sync.dma_start(
                      img_fm[:],
                      img_T[:, b0:b0 + BT].rearrange("(kc p) b -> p kc b", p=128),
                  )
                  imglin = imglin_pool.tile([128, OC, BT], F32)
                  for oc in range(OC):
                      ip = ps_pool.tile([128, BT], F32, tag="ps")
                      for kc in range(OC):
                          nc.tensor.matmul(
                              ip[:],
                              lhsT=w1t_sb[:, kc, oc * 128:(oc + 1) * 128],
                              rhs=img_fm[:, kc, :],
                              start=(kc == 0), stop=(kc == OC - 1),
                          )
                      nc.vector.tensor_scalar(
                          imglin[:, oc, :], ip[:], bias_sb[:, oc:oc + 1], None,
                          AluOpType.add,
                      )

                  acc = acc_pool.tile([128, NG, D], F32)
                  for q in range(A // 4):
                      a0 = q * 4
                      # one 2MB DMA: edge slabs for four attribute nodes
                      efm = efm_pool.tile([128, 4, OC, BT], F32R)
                      nc.sync.dma_start(
                          efm[:],
                          edge_T[a0:a0 + 4, :, b0:b0 + BT].rearrange(
                              "a (ic p) b -> p a ic b", p=128),
                      )
                      if bt == 0 and repeat is None:
                          nc.sync.dma_start(
                              rhs_sb[q][:],
                              rhs_cat[a0:a0 + 4].rearrange(
                                  "a (oc p) n -> p a oc n", p=128),
                          )
                      attr_sb = attr_pool.tile([128, NG, 4, D], F32)
                      for ai in range(4):
                          a = a0 + ai
                          # ---- stage 1: agg (feature-major) ----
                          agg = agg_pool.tile([128, OC, BT], F32R)
                          for oc in range(OC):
                              aps = ps_pool.tile([128, BT], F32, tag="ps")
                              for ic in range(OC):
                                  nc.tensor.matmul(
                                      aps[:],
                                      lhsT=w2t_sb[:, ic, oc * 128:(oc + 1) * 128],
                                      rhs=efm[:, ai, ic, :],
                                      start=(ic == 0), stop=(ic == OC - 1),
                                  )
                              # fused PSUM->SBUF move + img_lin add
                              nc.vector.tensor_tensor(
                                  agg[:, oc, :], aps[:], imglin[:, oc, :],
                                  AluOpType.add,
                              )
                          # ---- stage 2+3: attr | z  (batch-major) ----
                          for g in range(NG):
                              op = outps_pool.tile([128, 2 * D], F32, tag="outps")
                              for oc in range(OC):
                                  nc.tensor.matmul(
                                      op[:],
                                      lhsT=agg[:, oc, g * 128:(g + 1) * 128],
                                      rhs=rhs_sb[q][:, ai, oc, :],
                                      start=(oc == 0), stop=(oc == OC - 1),
                                  )
                              nc.scalar.copy(attr_sb[:, g, ai, :], op[:, 0:D])
                              z = op[:, D:2 * D]
                              if wmode == "ones":
                                  # w == 1: acc = max(z, 0) + acc in one DVE op
                                  if a == 0:
                                      nc.vector.tensor_scalar(
                                          acc[:, g, :], z, 0.0, None,
                                          AluOpType.max,
                                      )
                                  else:
                                      nc.vector.scalar_tensor_tensor(
                                          acc[:, g, :], z, 0.0, acc[:, g, :],
                                          AluOpType.max, AluOpType.add,
                                      )
                              elif g in act_groups:
                                  if a == 0:
                                      nc.scalar.activation(
                                          acc[:, g, :], z,
                                          mybir.ActivationFunctionType.Relu,
                                          scale=wvec_sb[:, a:a + 1],
                                      )
                                  else:
                                      t = tmp_pool.tile([128, D], F32, tag="tmp")
                                      nc.scalar.activation(
                                          t[:], z,
                                          mybir.ActivationFunctionType.Relu,
                                          scale=wvec_sb[:, a:a + 1],
                                      )
                                      nc.vector.tensor_tensor(
                                          acc[:, g, :], acc[:, g, :], t[:],
                                          AluOpType.add,
                                      )
                              else:
                                  if a == 0:
                                      nc.vector.tensor_scalar(
                                          acc[:, g, :], z, 0.0,
                                          wvec_sb[:, a:a + 1],
                                          AluOpType.max, AluOpType.mult,
                                      )
                                  else:
                                      t = tmp_pool.tile([128, D], F32, tag="tmp")
                                      nc.vector.tensor_scalar(
                                          t[:], z, 0.0, wvec_sb[:, a:a + 1],
                                          AluOpType.max, AluOpType.mult,
                                      )
                                      nc.vector.tensor_tensor(
                                          acc[:, g, :], acc[:, g, :], t[:],
                                          AluOpType.add,
                                      )
                      nc.sync.dma_start(
                          attr_o[b0:b0 + BT, a0:a0 + 4, :].rearrange(
                              "(g p) a d -> p g a d", p=128),
                          attr_sb[:],
                      )
                  nc.sync.dma_start(
                      ind_o[b0:b0 + BT, :].rearrange("(g p) d -> p g d", p=128),
                      acc[:],
                  )
    nc.compile()
    return nc


def _wmode(w):
    if bool((w == 1.0).all()):
        return "ones"
    if bool((w >= 0).all()):
        return "nonneg"
    return "general"


def _get_program(wmode):
    if wmode is True:  # back-compat for test harness
        wmode = "ones"
    if wmode not in _prog_cache:
        _prog_cache[wmode] = _build_program(wmode)
    return _prog_cache[wmode]


def kernel(image_features, edge_attributes, W_agg, b_agg, P_fwd, P_bwd,
           self_weighted):
    img = np.ascontiguousarray(np.asarray(image_features, dtype=np.float32))
    edge = np.ascontiguousarray(np.asarray(edge_attributes, dtype=np.float32))
    W = np.asarray(W_agg, dtype=np.float32)
    bvec = np.asarray(b_agg, dtype=np.float32)
    Pf = np.asarray(P_fwd, dtype=np.float32)
    Pb = np.asarray(P_bwd, dtype=np.float32)
    w = np.asarray(self_weighted, dtype=np.float32)[0, :, 0]

    nc = _get_program(_wmode(w))

    # shard batch across cores; repack activations feature-major
    edge_s = np.ascontiguousarray(
        edge.reshape(N_CORES, BL, A, D).transpose(0, 2, 3, 1))   # [8, A, D, BL]
    img_s = np.ascontiguousarray(
        img.reshape(N_CORES, BL, D).transpose(0, 2, 1))          # [8, D, BL]

    w1t = np.ascontiguousarray(W[:, :D].T)                       # [in k, out o]
    w2t = np.ascontiguousarray(W[:, D:].T)                       # [in i, out o]
    Pc = np.matmul(Pf.astype(np.float64), Pb.astype(np.float64)).astype(np.float32)
    rhs = np.ascontiguousarray(np.concatenate([Pf, Pc], axis=2)) # [A, D, 2D]
    bias2 = np.ascontiguousarray(bvec.reshape(OC, 128).T)        # [128, OC]
    wvec = np.ascontiguousarray(np.broadcast_to(w, (128, A)))    # [128, A]

    in_maps = [
        dict(edge_T=edge_s[c], img_T=img_s[c], w1t=w1t, w2t=w2t,
             rhs_cat=rhs, bias2=bias2, wvec=wvec)
        for c in range(N_CORES)
    ]
    res = run_bass_kernel_spmd(nc, in_maps, core_ids=list(range(N_CORES)))
    attr = np.concatenate([res.results[c]["attr_out"] for c in range(N_CORES)], axis=0)
    ind = np.concatenate([res.results[c]["ind_out"] for c in range(N_CORES)], axis=0)
    return attr, ind

